# revision 19
# baseline (speedup 1.0000x reference)
"""Trainium2 Bass kernel for causal multi-head attention (GPT-style block).

Reference computation (per batch b):
    qkv = x @ w_attn + b_attn ; q,k,v = split(qkv)
    per head: S = q k^T / sqrt(64); causal mask; P = softmax(S); a = P v
    out = concat_heads(a) @ w_proj + b_proj

Shapes: x (2, 2048, 1024), 16 heads, head_dim 64.

Sharding: 8 cores = 2 batches x 4 head-groups (4 heads each).  Tensor
parallel over heads: each core computes the QKV projection for its 4 heads
(column slice of w_attn), full causal attention for those heads, and its
partial output projection (row slice of w_proj).  Host sums the 4
head-group partials per batch and adds b_proj.

On-chip layouts (per core, T=2048, CW=256=4*64):
    xT      [1024, T]   x transposed (host-prepped bf16), streamed per chunk
    Q^T,K^T [CW, T]     projections with head-channel on partitions (bf16)
    V_aug   [T, 4*65]   V natural layout + ones column per head (the ones
                        column makes the PV matmul also produce the softmax
                        denominator as output row 64)
    S^T     [k, q]      scores transposed: PSUM [128, 512] per (k-tile,
                        q-chunk); P^T = exp(S^T/8) directly feeds PV as the
                        moving operand - no transposes in the hot loop.

All matmuls are bf16 (1 cycle/row; fp32r measured ~1.85 cyc/row on this
silicon).  Causal masking: off-band blocks are skipped; band blocks are
exp'd only on their live columns and zeroed above the diagonal with
gpsimd.affine_select (exact: exp(s)*0) on the otherwise idle GPSIMD
engine.

The attention inner loop is the only dependency-limited stretch (S ->
exp -> mask -> PV).  To keep the PE dense there (TRN2's HAM re-throttles
the PE clock to 1.2 GHz whenever an activity window contains idle), all
other matmul work - the NEXT chunk's QKV projection groups, V
transposes, and the PREVIOUS chunk's output projection - is queued as
"filler" ops and drained a few per attention step between the S and PV
matmuls.  PSUM banks: 0-3 S-blocks (lookahead 2 x 2 heads), 4/5 PV
accumulators for the head pair, 6/7 toggling filler accumulators.
"""

import sys

sys.path.insert(0, "/opt/trn_rl_repo")

import numpy as np
import ml_dtypes

import concourse.bacc as bacc
import concourse.mybir as mybir
import concourse.tile as tile
from concourse.bass_utils import run_bass_kernel_spmd

F32 = mybir.dt.float32
F32R = mybir.dt.float32r
BF16 = mybir.dt.bfloat16
NP_BF16 = np.dtype(ml_dtypes.bfloat16)

B = 2
T = 2048
NX = 1024
H = 16
HD = 64
NCORES = 8
NHG = 4          # head groups (cores per batch)
NH = 4           # heads per core
CW = NH * HD     # 256 channel width per core
QC = 512         # q-chunk (moving dim)
NQC = T // QC    # 4
KT = 128         # k-tile
VW = HD + 1      # 65: V columns + ones column


def _build():
    nc = bacc.Bacc("TRN2", target_bir_lowering=False, debug=False,
                   num_devices=NCORES)
    xT_d = nc.dram_tensor("xT", [NX, T], BF16, kind="ExternalInput")
    wq_d = nc.dram_tensor("wq", [NX, CW], BF16, kind="ExternalInput")
    wk_d = nc.dram_tensor("wk", [NX, CW], BF16, kind="ExternalInput")
    wv_d = nc.dram_tensor("wv", [NX, CW], BF16, kind="ExternalInput")
    bias_d = nc.dram_tensor("bias", [128, 6], F32, kind="ExternalInput")
    wp_d = nc.dram_tensor("wp", [CW, NX], BF16, kind="ExternalInput")
    ident_d = nc.dram_tensor("ident", [128, 128], F32R, kind="ExternalInput")
    vones_d = nc.dram_tensor("vones", [128, 16 * NH], BF16, kind="ExternalInput")
    out_d = nc.dram_tensor("out_p", [T, NX], F32, kind="ExternalOutput")

    Exp = mybir.ActivationFunctionType.Exp

    with tile.TileContext(nc) as tc:
        with (
            tc.tile_pool(name="pers", bufs=1) as pers,
            tc.tile_pool(name="xin", bufs=4) as xin,
            tc.tile_pool(name="ps", bufs=1, space="PSUM") as psum,
            tc.tile_pool(name="ptp", bufs=8) as ptp,
            tc.tile_pool(name="stg", bufs=4) as stg,
            tc.tile_pool(name="op", bufs=4) as op,
            tc.tile_pool(name="rp", bufs=4) as rp,
        ):
            def bank(i, shape, dtype=F32):
                return psum.tile(shape, dtype, tag=f"bank{i}", bufs=1,
                                 name=f"bank{i}")

            # ---- persistent tiles; load order: first-needed first ----
            # ones columns of V_aug first: tiny DMA + DVE fill (element-exact;
            # a sub-512B strided DMA would read-modify-write and race the V
            # data copies) so the first PV matmuls never wait on it
            vaug = pers.tile([128, T // KT, NH * VW], BF16, tag="vaug")
            vones_sb = pers.tile([128, 16 * NH], BF16, tag="vones")
            nc.sync.dma_start(vones_sb[:], vones_d.ap())
            nc.vector.tensor_copy(
                vaug[:].rearrange("p t (h w) -> p t h w", h=NH)[:, :, :, HD:HD + 1],
                vones_sb[:].rearrange("p (t h w) -> p t h w", t=16, h=NH),
            )
            wqkv = pers.tile([128, 8, 3 * CW], BF16, tag="wqkv")
            for j in range(8):
                for w_d, off in ((wq_d, 0), (wk_d, CW), (wv_d, 2 * CW)):
                    nc.sync.dma_start(
                        wqkv[:, j, off:off + CW],
                        w_d.ap().rearrange("(j p) c -> j p c", p=128)[j],
                    )
            xT_r = xT_d.ap().rearrange("(j p) t -> j p t", p=128)

            def load_x(qq):
                xt = xin.tile([128, 8, QC], BF16, tag="xt")
                for j in range(8):
                    nc.sync.dma_start(xt[:, j, :],
                                      xT_r[j][:, qq * QC:(qq + 1) * QC])
                return xt

            xt0 = load_x(0)
            xts = {qq: load_x(qq) for qq in range(1, NQC)}
            bias = pers.tile([128, 6], F32, tag="bias")
            nc.sync.dma_start(bias[:], bias_d.ap())
            ident = pers.tile([128, 128], F32R, tag="ident")
            nc.sync.dma_start(ident[:], ident_d.ap())

            wp = pers.tile([128, 2, NX], BF16, tag="wp")
            nc.sync.dma_start(wp[:], wp_d.ap().rearrange("(c p) n -> p c n", p=128))

            QT = [pers.tile([128, T], BF16, tag=f"qt{i}", name=f"qt{i}")
                  for i in range(2)]
            KTs = [pers.tile([128, T], BF16, tag=f"kt{i}", name=f"kt{i}")
                   for i in range(2)]
            anorm = [pers.tile([128, T], BF16, tag=f"an{i}", name=f"an{i}")
                     for i in range(2)]

            fl_state = {"toggle": 0, "bank": None}

            def fl_bank(shape, dtype=F32):
                fl_state["toggle"] ^= 1
                fl_state["bank"] = bank(6 + fl_state["toggle"], shape, dtype)
                return fl_state["bank"]

            def v_transpose_ops(qq, vstages):
                """Filler ops: PE-transpose V^T chunk -> V natural in vaug."""
                ops = []
                for c2 in range(2):
                    for blk in range(4):
                        def f(c2=c2, blk=blk):
                            vs = vstages[c2]
                            pt_ps = fl_bank([128, 128], F32R)
                            nc.tensor.transpose(
                                pt_ps[:], vs[:, blk * 128:(blk + 1) * 128],
                                ident[:])
                            tt = qq * 4 + blk
                            dst = vaug[:, tt,
                                       c2 * 2 * VW:c2 * 2 * VW + 2 * VW]
                            dst = dst.rearrange("p (h w) -> p h w",
                                                h=2)[:, :, 0:HD]
                            src = pt_ps[:].rearrange("p (h w) -> p h w", h=2)
                            nc.vector.tensor_copy(dst, src)
                        ops.append(f)
                return ops

            def qkv_ops(qq, xt):
                """Filler ops: QKV projection for chunk qq.
                6 groups (q/k/v x c2-half) of 8 accumulating matmuls into a
                toggling filler bank; evac on DVE (with bias); V groups are
                followed by their transpose fillers."""
                cs = slice(qq * QC, (qq + 1) * QC)
                vstages = [None, None]
                ops = []
                for off, kind in ((0, "q"), (CW, "k"), (2 * CW, "v")):
                    for c2 in range(2):
                        for j in range(8):
                            def f(off=off, kind=kind, c2=c2, j=j):
                                if j == 0:
                                    fl_bank([128, QC])
                                g = fl_state["bank"]
                                lhsT = wqkv[:, j,
                                            off + c2 * 128:off + (c2 + 1) * 128]
                                nc.tensor.matmul(g[:], lhsT, xt[:, j, :],
                                                 start=(j == 0), stop=(j == 7))
                                if j == 7:
                                    bcol = {"q": 0, "k": 2, "v": 4}[kind] + c2
                                    bap = bias[:, bcol:bcol + 1]
                                    if kind == "q":
                                        nc.vector.tensor_scalar_add(
                                            QT[c2][:, cs], g[:], bap)
                                    elif kind == "k":
                                        nc.vector.tensor_scalar_add(
                                            KTs[c2][:, cs], g[:], bap)
                                    else:
                                        vs = stg.tile([128, QC], F32R,
                                                      tag="vstage")
                                        nc.vector.tensor_scalar_add(
                                            vs[:], g[:], bap)
                                        vstages[c2] = vs
                            ops.append(f)
                ops += v_transpose_ops(qq, vstages)
                return ops

            def cproj_ops(qq):
                """Filler ops: output projection for t-rows of chunk qq."""
                ops = []
                for i in range(4):
                    tt = qq * 4 + i
                    for nxc in range(2):
                        for c2 in range(2):
                            def f(tt=tt, nxc=nxc, c2=c2, i=i):
                                if c2 == 0:
                                    fl_bank([128, QC])
                                po = fl_state["bank"]
                                nc.tensor.matmul(
                                    po[:],
                                    anorm[c2][:, tt * 128:(tt + 1) * 128],
                                    wp[:, c2, nxc * QC:(nxc + 1) * QC],
                                    start=(c2 == 0), stop=(c2 == 1))
                                if c2 == 1:
                                    ot = op.tile([128, QC], F32, tag="ot")
                                    if (i * 2 + nxc) % 2 == 0:
                                        nc.scalar.copy(ot[:], po[:])
                                    else:
                                        nc.vector.tensor_copy(ot[:], po[:])
                                    nc.sync.dma_start(
                                        out_d.ap()[tt * 128:(tt + 1) * 128,
                                                   nxc * QC:(nxc + 1) * QC],
                                        ot[:])
                            ops.append(f)
                return ops

            def attention_pair(hp, qq, fillers, steps_left):
                """Heads (2hp, 2hp+1) for q-chunk qq, S/PV interleaved with
                filler drain (spread adaptively over remaining steps)."""
                c2 = hp
                nk = 4 * qq + 4
                qs = slice(qq * QC, (qq + 1) * QC)
                pa = [bank(4, [VW, QC]), bank(5, [VW, QC])]
                pts = {}
                LA = 2

                def s_block(kk, hh):
                    ps_s = bank((2 * kk + hh) % 4, [128, QC])
                    rows = slice(64 * hh, 64 * hh + 64)
                    lhsT = KTs[c2][rows, kk * KT:(kk + 1) * KT]
                    rhs = QT[c2][rows, qs]
                    nc.tensor.matmul(ps_s[:], lhsT, rhs, start=True, stop=True)
                    pt = ptp.tile([128, QC], BF16, tag="pt")
                    if kk >= 4 * qq:
                        # band block: columns < 128j are fully masked - skip
                        # their exp; affine_select zero-fills them plus the
                        # above-diagonal triangle of the next 128 columns
                        j = kk - 4 * qq
                        nc.scalar.activation(pt[:, 128 * j:QC],
                                             ps_s[:, 128 * j:QC], Exp,
                                             scale=0.125)
                        if j > 0:
                            nc.gpsimd.memset(pt[:, 0:128 * j], 0.0)
                        nc.gpsimd.affine_select(
                            pt[:, 128 * j:128 * (j + 1)],
                            pt[:, 128 * j:128 * (j + 1)],
                            pattern=[[1, 128]],
                            compare_op=mybir.AluOpType.is_ge, fill=0.0,
                            base=0, channel_multiplier=-1)
                    else:
                        nc.scalar.activation(pt[:], ps_s[:], Exp, scale=0.125)
                    pts[(kk, hh)] = pt

                def pv_block(kk, hh):
                    h = 2 * hp + hh
                    lhsT = vaug[:, kk, h * VW:(h + 1) * VW]
                    nc.tensor.matmul(pa[hh][:], lhsT, pts.pop((kk, hh))[:],
                                     start=(kk == 0), stop=(kk == nk - 1))

                for kk in range(min(LA, nk)):
                    s_block(kk, 0)
                    s_block(kk, 1)
                for kk in range(nk):
                    if kk + LA < nk:
                        s_block(kk + LA, 0)
                        s_block(kk + LA, 1)
                    n = -(-len(fillers) // max(1, steps_left[0]))
                    steps_left[0] -= 1
                    for _ in range(n):
                        if fillers:
                            fillers.pop(0)()
                    pv_block(kk, 0)
                    pv_block(kk, 1)

                for hh in range(2):
                    rows = slice(64 * hh, 64 * hh + 64)
                    dn = rp.tile([1, QC], F32, tag="dn")
                    nc.vector.tensor_copy(dn[:], pa[hh][HD:HD + 1, :])
                    recip = rp.tile([1, QC], F32, tag="recip")
                    nc.vector.reciprocal_approx_fast(recip[:], dn[:])
                    rbc = rp.tile([64, QC], F32, tag="rbc")
                    nc.gpsimd.partition_broadcast(rbc[:], recip[:])
                    nc.vector.tensor_mul(anorm[c2][rows, qs],
                                         pa[hh][0:HD, :], rbc[:])

            # ---- main pipeline over q-chunks ----
            # chunk 0 QKV runs dense up front; each attention stretch then
            # drains the next chunk's QKV + previous chunk's c_proj as
            # fillers between its S and PV matmuls.
            for f in qkv_ops(0, xt0):
                f()
            for qq in range(NQC):
                fillers = []
                if qq + 1 < NQC:
                    fillers += qkv_ops(qq + 1, xts[qq + 1])
                if qq >= 1:
                    fillers += cproj_ops(qq - 1)
                steps_left = [2 * (4 * qq + 4)]
                for hp in range(2):
                    attention_pair(hp, qq, fillers, steps_left)
                while fillers:
                    fillers.pop(0)()
            for f in cproj_ops(NQC - 1):
                f()

    nc.compile()
    return nc


_CACHE = {}


def _get_nc():
    if "nc" not in _CACHE:
        _CACHE["nc"] = _build()
    return _CACHE["nc"]


def kernel(x, w_attn, b_attn, w_proj, b_proj):
    x = np.asarray(x, dtype=np.float32)
    w_attn = np.asarray(w_attn, dtype=np.float32)
    b_attn = np.asarray(b_attn, dtype=np.float32)
    w_proj = np.asarray(w_proj, dtype=np.float32)
    b_proj = np.asarray(b_proj, dtype=np.float32)

    ident = np.eye(128, dtype=np.float32)
    vones = np.ones((128, 64), dtype=NP_BF16)
    in_maps = []
    for core in range(NCORES):
        b, hg = divmod(core, NHG)
        cols = slice(hg * CW, (hg + 1) * CW)
        bias = np.empty((128, 6), dtype=np.float32)
        for qkv_i in range(3):
            bseg = b_attn[qkv_i * NX:][cols]
            bias[:, 2 * qkv_i] = bseg[:128]
            bias[:, 2 * qkv_i + 1] = bseg[128:]
        in_maps.append({
            "xT": np.ascontiguousarray(x[b].T).astype(NP_BF16),
            "wq": np.ascontiguousarray(w_attn[:, cols]).astype(NP_BF16),
            "wk": np.ascontiguousarray(w_attn[:, NX:][:, cols]).astype(NP_BF16),
            "wv": np.ascontiguousarray(w_attn[:, 2 * NX:][:, cols]).astype(NP_BF16),
            "bias": bias,
            "wp": np.ascontiguousarray(w_proj[cols, :]).astype(NP_BF16),
            "ident": ident,
            "vones": vones,
        })

    nc = _get_nc()
    res = run_bass_kernel_spmd(nc, in_maps, core_ids=list(range(NCORES)))
    _CACHE["last_res"] = res
    out = np.empty((B, T, NX), dtype=np.float32)
    for b in range(B):
        acc = res.results[b * NHG]["out_p"].astype(np.float32)
        for hg in range(1, NHG):
            acc = acc + res.results[b * NHG + hg]["out_p"]
        out[b] = acc + b_proj
    return out


# revision 20
# speedup vs baseline: 1.0456x; 1.0456x over previous
"""Trainium2 Bass kernel for causal multi-head attention (GPT-style block).

Reference computation (per batch b):
    qkv = x @ w_attn + b_attn ; q,k,v = split(qkv)
    per head: S = q k^T / sqrt(64); causal mask; P = softmax(S); a = P v
    out = concat_heads(a) @ w_proj + b_proj

Shapes: x (2, 2048, 1024), 16 heads, head_dim 64.

Sharding: 8 cores = 2 batches x 4 head-groups (4 heads each).  Tensor
parallel over heads: each core computes the QKV projection for its 4 heads
(column slice of w_attn), full causal attention for those heads, and its
partial output projection (row slice of w_proj).  Host sums the 4
head-group partials per batch and adds b_proj.

On-chip layouts (per core, T=2048, CW=256=4*64):
    xT      [1024, T]   x transposed (host-prepped bf16), streamed per chunk
    Q^T,K^T [CW, T]     projections with head-channel on partitions (bf16)
    V_aug   [T, 4*65]   V natural layout + ones column per head (the ones
                        column makes the PV matmul also produce the softmax
                        denominator as output row 64)
    S^T     [k, q]      scores transposed: PSUM [128, 512] per (k-tile,
                        q-chunk); P^T = exp(S^T/8) directly feeds PV as the
                        moving operand - no transposes in the hot loop.

All matmuls are bf16 (1 cycle/row; fp32r measured ~1.85 cyc/row on this
silicon).  Causal masking: off-band blocks are skipped; band blocks are
exp'd only on their live columns and zeroed above the diagonal with
gpsimd.affine_select (exact: exp(s)*0) on the otherwise idle GPSIMD
engine.

The attention inner loop is the only dependency-limited stretch (S ->
exp -> mask -> PV).  To keep the PE dense there (TRN2's HAM re-throttles
the PE clock to 1.2 GHz whenever an activity window contains idle), all
other matmul work - the NEXT chunk's QKV projection groups, V
transposes, and the PREVIOUS chunk's output projection - is queued as
"filler" ops and drained a few per attention step between the S and PV
matmuls.  PSUM banks: 0-3 S-blocks (lookahead 2 x 2 heads), 4/5 PV
accumulators for the head pair, 6/7 toggling filler accumulators.
"""

import sys

sys.path.insert(0, "/opt/trn_rl_repo")

import numpy as np
import ml_dtypes

import concourse.bacc as bacc
import concourse.mybir as mybir
import concourse.tile as tile
from concourse.bass_utils import run_bass_kernel_spmd

F32 = mybir.dt.float32
F32R = mybir.dt.float32r
BF16 = mybir.dt.bfloat16
NP_BF16 = np.dtype(ml_dtypes.bfloat16)

B = 2
T = 2048
NX = 1024
H = 16
HD = 64
NCORES = 8
NHG = 4          # head groups (cores per batch)
NH = 4           # heads per core
CW = NH * HD     # 256 channel width per core
QC = 512         # q-chunk (moving dim)
NQC = T // QC    # 4
KT = 128         # k-tile
VW = HD + 1      # 65: V columns + ones column


def _build():
    nc = bacc.Bacc("TRN2", target_bir_lowering=False, debug=False,
                   num_devices=NCORES)
    xT_d = nc.dram_tensor("xT", [NX, T], BF16, kind="ExternalInput")
    wq_d = nc.dram_tensor("wq", [NX, CW], BF16, kind="ExternalInput")
    wk_d = nc.dram_tensor("wk", [NX, CW], BF16, kind="ExternalInput")
    wv_d = nc.dram_tensor("wv", [NX, CW], BF16, kind="ExternalInput")
    bias_d = nc.dram_tensor("bias", [128, 6], F32, kind="ExternalInput")
    wp_d = nc.dram_tensor("wp", [CW, NX], BF16, kind="ExternalInput")
    ident_d = nc.dram_tensor("ident", [128, 128], F32R, kind="ExternalInput")
    vones_d = nc.dram_tensor("vones", [128, 16 * NH], BF16, kind="ExternalInput")
    out_d = nc.dram_tensor("out_p", [T, NX], F32, kind="ExternalOutput")

    Exp = mybir.ActivationFunctionType.Exp

    with tile.TileContext(nc) as tc:
        with (
            tc.tile_pool(name="pers", bufs=1) as pers,
            tc.tile_pool(name="xin", bufs=4) as xin,
            tc.tile_pool(name="ps", bufs=1, space="PSUM") as psum,
            tc.tile_pool(name="ptp", bufs=8) as ptp,
            tc.tile_pool(name="stg", bufs=4) as stg,
            tc.tile_pool(name="op", bufs=4) as op,
            tc.tile_pool(name="rp", bufs=4) as rp,
        ):
            def bank(i, shape, dtype=F32):
                return psum.tile(shape, dtype, tag=f"bank{i}", bufs=1,
                                 name=f"bank{i}")

            # ---- persistent tiles; load order: first-needed first ----
            # ones columns of V_aug first: tiny DMA + DVE fill (element-exact;
            # a sub-512B strided DMA would read-modify-write and race the V
            # data copies) so the first PV matmuls never wait on it
            vaug = pers.tile([128, T // KT, NH * VW], BF16, tag="vaug")
            vones_sb = pers.tile([128, 16 * NH], BF16, tag="vones")
            nc.sync.dma_start(vones_sb[:], vones_d.ap())
            nc.vector.tensor_copy(
                vaug[:].rearrange("p t (h w) -> p t h w", h=NH)[:, :, :, HD:HD + 1],
                vones_sb[:].rearrange("p (t h w) -> p t h w", t=16, h=NH),
            )
            bias = pers.tile([128, 6], F32, tag="bias")
            nc.sync.dma_start(bias[:], bias_d.ap())
            ident = pers.tile([128, 128], F32R, tag="ident")
            nc.sync.dma_start(ident[:], ident_d.ap())
            wqkv = pers.tile([128, 8, 3 * CW], BF16, tag="wqkv")
            for w_d, off in ((wq_d, 0), (wk_d, CW), (wv_d, 2 * CW)):
                nc.sync.dma_start(
                    wqkv[:, :, off:off + CW],
                    w_d.ap().rearrange("(j p) c -> p j c", p=128),
                )
            xT_r = xT_d.ap().rearrange("(j p) t -> j p t", p=128)

            def load_x(qq):
                xt = xin.tile([128, 8, QC], BF16, tag="xt")
                for j in range(8):
                    nc.sync.dma_start(xt[:, j, :],
                                      xT_r[j][:, qq * QC:(qq + 1) * QC])
                return xt

            xt0 = load_x(0)
            xts = {qq: load_x(qq) for qq in range(1, NQC)}
            bias = pers.tile([128, 6], F32, tag="bias")
            nc.sync.dma_start(bias[:], bias_d.ap())
            ident = pers.tile([128, 128], F32R, tag="ident")
            nc.sync.dma_start(ident[:], ident_d.ap())

            wp = pers.tile([128, 2, NX], BF16, tag="wp")
            nc.sync.dma_start(wp[:], wp_d.ap().rearrange("(c p) n -> p c n", p=128))

            QT = [pers.tile([128, T], BF16, tag=f"qt{i}", name=f"qt{i}")
                  for i in range(2)]
            KTs = [pers.tile([128, T], BF16, tag=f"kt{i}", name=f"kt{i}")
                   for i in range(2)]
            anorm = [pers.tile([128, T], BF16, tag=f"an{i}", name=f"an{i}")
                     for i in range(2)]

            fl_state = {"toggle": 0, "bank": None}

            def fl_bank(shape, dtype=F32):
                fl_state["toggle"] ^= 1
                fl_state["bank"] = bank(6 + fl_state["toggle"], shape, dtype)
                return fl_state["bank"]

            def v_transpose_ops(qq, vstages):
                """Filler ops: PE-transpose V^T chunk -> V natural in vaug."""
                ops = []
                for c2 in range(2):
                    for blk in range(4):
                        def f(c2=c2, blk=blk):
                            vs = vstages[c2]
                            pt_ps = fl_bank([128, 128], F32R)
                            nc.tensor.transpose(
                                pt_ps[:], vs[:, blk * 128:(blk + 1) * 128],
                                ident[:])
                            tt = qq * 4 + blk
                            dst = vaug[:, tt,
                                       c2 * 2 * VW:c2 * 2 * VW + 2 * VW]
                            dst = dst.rearrange("p (h w) -> p h w",
                                                h=2)[:, :, 0:HD]
                            src = pt_ps[:].rearrange("p (h w) -> p h w", h=2)
                            nc.vector.tensor_copy(dst, src)
                        ops.append(f)
                return ops

            def qkv_ops(qq, xt):
                """Filler ops: QKV projection for chunk qq.
                6 groups (q/k/v x c2-half) of 8 accumulating matmuls into a
                toggling filler bank; evac on DVE (with bias); V groups are
                followed by their transpose fillers."""
                cs = slice(qq * QC, (qq + 1) * QC)
                vstages = [None, None]
                ops = []
                for off, kind in ((0, "q"), (CW, "k"), (2 * CW, "v")):
                    for c2 in range(2):
                        for j in range(8):
                            def f(off=off, kind=kind, c2=c2, j=j):
                                if j == 0:
                                    fl_bank([128, QC])
                                g = fl_state["bank"]
                                lhsT = wqkv[:, j,
                                            off + c2 * 128:off + (c2 + 1) * 128]
                                nc.tensor.matmul(g[:], lhsT, xt[:, j, :],
                                                 start=(j == 0), stop=(j == 7))
                                if j == 7:
                                    bcol = {"q": 0, "k": 2, "v": 4}[kind] + c2
                                    bap = bias[:, bcol:bcol + 1]
                                    if kind == "q":
                                        nc.vector.tensor_scalar_add(
                                            QT[c2][:, cs], g[:], bap)
                                    elif kind == "k":
                                        nc.vector.tensor_scalar_add(
                                            KTs[c2][:, cs], g[:], bap)
                                    else:
                                        vs = stg.tile([128, QC], F32R,
                                                      tag="vstage")
                                        nc.vector.tensor_scalar_add(
                                            vs[:], g[:], bap)
                                        vstages[c2] = vs
                            ops.append(f)
                ops += v_transpose_ops(qq, vstages)
                return ops

            def cproj_ops(qq):
                """Filler ops: output projection for t-rows of chunk qq."""
                ops = []
                for i in range(4):
                    tt = qq * 4 + i
                    for nxc in range(2):
                        for c2 in range(2):
                            def f(tt=tt, nxc=nxc, c2=c2, i=i):
                                if c2 == 0:
                                    fl_bank([128, QC])
                                po = fl_state["bank"]
                                nc.tensor.matmul(
                                    po[:],
                                    anorm[c2][:, tt * 128:(tt + 1) * 128],
                                    wp[:, c2, nxc * QC:(nxc + 1) * QC],
                                    start=(c2 == 0), stop=(c2 == 1))
                                if c2 == 1:
                                    ot = op.tile([128, QC], F32, tag="ot")
                                    if (i * 2 + nxc) % 2 == 0:
                                        nc.scalar.copy(ot[:], po[:])
                                    else:
                                        nc.vector.tensor_copy(ot[:], po[:])
                                    nc.sync.dma_start(
                                        out_d.ap()[tt * 128:(tt + 1) * 128,
                                                   nxc * QC:(nxc + 1) * QC],
                                        ot[:])
                            ops.append(f)
                return ops

            def attention_pair(hp, qq, fillers, steps_left):
                """Heads (2hp, 2hp+1) for q-chunk qq, S/PV interleaved with
                filler drain (spread adaptively over remaining steps)."""
                c2 = hp
                nk = 4 * qq + 4
                qs = slice(qq * QC, (qq + 1) * QC)
                pa = [bank(4, [VW, QC]), bank(5, [VW, QC])]
                pts = {}
                LA = 2

                def s_block(kk, hh):
                    ps_s = bank((2 * kk + hh) % 4, [128, QC])
                    rows = slice(64 * hh, 64 * hh + 64)
                    lhsT = KTs[c2][rows, kk * KT:(kk + 1) * KT]
                    rhs = QT[c2][rows, qs]
                    nc.tensor.matmul(ps_s[:], lhsT, rhs, start=True, stop=True)
                    pt = ptp.tile([128, QC], BF16, tag="pt")
                    if kk >= 4 * qq:
                        # band block: columns < 128j are fully masked - skip
                        # their exp; affine_select zero-fills them plus the
                        # above-diagonal triangle of the next 128 columns
                        j = kk - 4 * qq
                        nc.scalar.activation(pt[:, 128 * j:QC],
                                             ps_s[:, 128 * j:QC], Exp,
                                             scale=0.125)
                        if j > 0:
                            nc.gpsimd.memset(pt[:, 0:128 * j], 0.0)
                        nc.gpsimd.affine_select(
                            pt[:, 128 * j:128 * (j + 1)],
                            pt[:, 128 * j:128 * (j + 1)],
                            pattern=[[1, 128]],
                            compare_op=mybir.AluOpType.is_ge, fill=0.0,
                            base=0, channel_multiplier=-1)
                    else:
                        nc.scalar.activation(pt[:], ps_s[:], Exp, scale=0.125)
                    pts[(kk, hh)] = pt

                def pv_block(kk, hh):
                    h = 2 * hp + hh
                    lhsT = vaug[:, kk, h * VW:(h + 1) * VW]
                    nc.tensor.matmul(pa[hh][:], lhsT, pts.pop((kk, hh))[:],
                                     start=(kk == 0), stop=(kk == nk - 1))

                for kk in range(min(LA, nk)):
                    s_block(kk, 0)
                    s_block(kk, 1)
                for kk in range(nk):
                    if kk + LA < nk:
                        s_block(kk + LA, 0)
                        s_block(kk + LA, 1)
                    n = -(-len(fillers) // max(1, steps_left[0]))
                    steps_left[0] -= 1
                    for _ in range(n):
                        if fillers:
                            fillers.pop(0)()
                    pv_block(kk, 0)
                    pv_block(kk, 1)

                for hh in range(2):
                    rows = slice(64 * hh, 64 * hh + 64)
                    dn = rp.tile([1, QC], F32, tag="dn")
                    nc.vector.tensor_copy(dn[:], pa[hh][HD:HD + 1, :])
                    recip = rp.tile([1, QC], F32, tag="recip")
                    nc.vector.reciprocal_approx_fast(recip[:], dn[:])
                    rbc = rp.tile([64, QC], F32, tag="rbc")
                    nc.gpsimd.partition_broadcast(rbc[:], recip[:])
                    nc.vector.tensor_mul(anorm[c2][rows, qs],
                                         pa[hh][0:HD, :], rbc[:])

            # ---- main pipeline over q-chunks ----
            # chunk 0 QKV runs dense up front; each attention stretch then
            # drains the next chunk's QKV + previous chunk's c_proj as
            # fillers between its S and PV matmuls.
            for f in qkv_ops(0, xt0):
                f()
            for qq in range(NQC):
                fillers = []
                if qq + 1 < NQC:
                    fillers += qkv_ops(qq + 1, xts[qq + 1])
                if qq >= 1:
                    fillers += cproj_ops(qq - 1)
                steps_left = [2 * (4 * qq + 4)]
                for hp in range(2):
                    attention_pair(hp, qq, fillers, steps_left)
                while fillers:
                    fillers.pop(0)()
            for f in cproj_ops(NQC - 1):
                f()

    nc.compile()
    return nc


_CACHE = {}


def _get_nc():
    if "nc" not in _CACHE:
        _CACHE["nc"] = _build()
    return _CACHE["nc"]


def kernel(x, w_attn, b_attn, w_proj, b_proj):
    x = np.asarray(x, dtype=np.float32)
    w_attn = np.asarray(w_attn, dtype=np.float32)
    b_attn = np.asarray(b_attn, dtype=np.float32)
    w_proj = np.asarray(w_proj, dtype=np.float32)
    b_proj = np.asarray(b_proj, dtype=np.float32)

    ident = np.eye(128, dtype=np.float32)
    vones = np.ones((128, 64), dtype=NP_BF16)
    in_maps = []
    for core in range(NCORES):
        b, hg = divmod(core, NHG)
        cols = slice(hg * CW, (hg + 1) * CW)
        bias = np.empty((128, 6), dtype=np.float32)
        for qkv_i in range(3):
            bseg = b_attn[qkv_i * NX:][cols]
            bias[:, 2 * qkv_i] = bseg[:128]
            bias[:, 2 * qkv_i + 1] = bseg[128:]
        in_maps.append({
            "xT": np.ascontiguousarray(x[b].T).astype(NP_BF16),
            "wq": np.ascontiguousarray(w_attn[:, cols]).astype(NP_BF16),
            "wk": np.ascontiguousarray(w_attn[:, NX:][:, cols]).astype(NP_BF16),
            "wv": np.ascontiguousarray(w_attn[:, 2 * NX:][:, cols]).astype(NP_BF16),
            "bias": bias,
            "wp": np.ascontiguousarray(w_proj[cols, :]).astype(NP_BF16),
            "ident": ident,
            "vones": vones,
        })

    nc = _get_nc()
    res = run_bass_kernel_spmd(nc, in_maps, core_ids=list(range(NCORES)))
    _CACHE["last_res"] = res
    out = np.empty((B, T, NX), dtype=np.float32)
    for b in range(B):
        acc = res.results[b * NHG]["out_p"].astype(np.float32)
        for hg in range(1, NHG):
            acc = acc + res.results[b * NHG + hg]["out_p"]
        out[b] = acc + b_proj
    return out


# revision 21
# speedup vs baseline: 1.0748x; 1.0279x over previous
"""Trainium2 Bass kernel for causal multi-head attention (GPT-style block).

Reference computation (per batch b):
    qkv = x @ w_attn + b_attn ; q,k,v = split(qkv)
    per head: S = q k^T / sqrt(64); causal mask; P = softmax(S); a = P v
    out = concat_heads(a) @ w_proj + b_proj

Shapes: x (2, 2048, 1024), 16 heads, head_dim 64.

Sharding: 8 cores = 2 batches x 4 head-groups (4 heads each).  Tensor
parallel over heads: each core computes the QKV projection for its 4 heads
(column slice of w_attn), full causal attention for those heads, and its
partial output projection (row slice of w_proj).  Host sums the 4
head-group partials per batch and adds b_proj.

On-chip layouts (per core, T=2048, CW=256=4*64):
    xT      [1024, T]   x transposed (host-prepped bf16), streamed per chunk
    Q^T,K^T [CW, T]     projections with head-channel on partitions (bf16)
    V_aug   [T, 4*65]   V natural layout + ones column per head (the ones
                        column makes the PV matmul also produce the softmax
                        denominator as output row 64)
    S^T     [k, q]      scores transposed: PSUM [128, 512] per (k-tile,
                        q-chunk); P^T = exp(S^T/8) directly feeds PV as the
                        moving operand - no transposes in the hot loop.

All matmuls are bf16 (1 cycle/row; fp32r measured ~1.85 cyc/row on this
silicon).  Causal masking: off-band blocks are skipped; band blocks are
exp'd only on their live columns and zeroed above the diagonal with
gpsimd.affine_select (exact: exp(s)*0) on the otherwise idle GPSIMD
engine.

The attention inner loop is the only dependency-limited stretch (S ->
exp -> mask -> PV).  To keep the PE dense there (TRN2's HAM re-throttles
the PE clock to 1.2 GHz whenever an activity window contains idle), all
other matmul work - the NEXT chunk's QKV projection groups, V
transposes, and the PREVIOUS chunk's output projection - is queued as
"filler" ops and drained a few per attention step between the S and PV
matmuls.  PSUM banks: 0-3 S-blocks (lookahead 2 x 2 heads), 4/5 PV
accumulators for the head pair, 6/7 toggling filler accumulators.
"""

import sys

sys.path.insert(0, "/opt/trn_rl_repo")

import numpy as np
import ml_dtypes

import concourse.bacc as bacc
import concourse.mybir as mybir
import concourse.tile as tile
from concourse.bass_utils import run_bass_kernel_spmd

F32 = mybir.dt.float32
F32R = mybir.dt.float32r
BF16 = mybir.dt.bfloat16
NP_BF16 = np.dtype(ml_dtypes.bfloat16)

B = 2
T = 2048
NX = 1024
H = 16
HD = 64
NCORES = 8
NHG = 4          # head groups (cores per batch)
NH = 4           # heads per core
CW = NH * HD     # 256 channel width per core
QC = 512         # q-chunk (moving dim)
NQC = T // QC    # 4
KT = 128         # k-tile
VW = HD + 1      # 65: V columns + ones column


def _build():
    nc = bacc.Bacc("TRN2", target_bir_lowering=False, debug=False,
                   num_devices=NCORES)
    xT_d = nc.dram_tensor("xT", [NX, T], BF16, kind="ExternalInput")
    wqkv_d = nc.dram_tensor("wqkv", [NX, 3 * CW], BF16, kind="ExternalInput")
    bias_d = nc.dram_tensor("bias", [128, 6], F32, kind="ExternalInput")
    wp_d = nc.dram_tensor("wp", [CW, NX], BF16, kind="ExternalInput")
    ident_d = nc.dram_tensor("ident", [128, 128], F32R, kind="ExternalInput")
    vones_d = nc.dram_tensor("vones", [128, 16 * NH], BF16, kind="ExternalInput")
    out_d = nc.dram_tensor("out_p", [T, NX], F32, kind="ExternalOutput")

    Exp = mybir.ActivationFunctionType.Exp

    with tile.TileContext(nc) as tc:
        with (
            tc.tile_pool(name="pers", bufs=1) as pers,
            tc.tile_pool(name="xin", bufs=4) as xin,
            tc.tile_pool(name="ps", bufs=1, space="PSUM") as psum,
            tc.tile_pool(name="ptp", bufs=8) as ptp,
            tc.tile_pool(name="stg", bufs=4) as stg,
            tc.tile_pool(name="op", bufs=4) as op,
            tc.tile_pool(name="rp", bufs=4) as rp,
        ):
            def bank(i, shape, dtype=F32):
                return psum.tile(shape, dtype, tag=f"bank{i}", bufs=1,
                                 name=f"bank{i}")

            # ---- persistent tiles; load order: first-needed first ----
            # ones columns of V_aug first: tiny DMA + DVE fill (element-exact;
            # a sub-512B strided DMA would read-modify-write and race the V
            # data copies) so the first PV matmuls never wait on it
            vaug = pers.tile([128, T // KT, NH * VW], BF16, tag="vaug")
            vones_sb = pers.tile([128, 16 * NH], BF16, tag="vones")
            nc.sync.dma_start(vones_sb[:], vones_d.ap())
            nc.vector.tensor_copy(
                vaug[:].rearrange("p t (h w) -> p t h w", h=NH)[:, :, :, HD:HD + 1],
                vones_sb[:].rearrange("p (t h w) -> p t h w", t=16, h=NH),
            )
            bias = pers.tile([128, 6], F32, tag="bias")
            nc.sync.dma_start(bias[:], bias_d.ap())
            ident = pers.tile([128, 128], F32R, tag="ident")
            nc.sync.dma_start(ident[:], ident_d.ap())
            wqkv = pers.tile([128, 8, 3 * CW], BF16, tag="wqkv")
            nc.sync.dma_start(wqkv[:],
                              wqkv_d.ap().rearrange("(j p) c -> p j c", p=128))
            def load_x(qq):
                xt = xin.tile([128, 8, QC], BF16, tag="xt")
                nc.gpsimd.dma_start(
                    xt[:],
                    xT_d.ap().rearrange("(j p) t -> p j t",
                                        p=128)[:, :, qq * QC:(qq + 1) * QC])
                return xt

            xt0 = load_x(0)
            xts = {qq: load_x(qq) for qq in range(1, NQC)}
            bias = pers.tile([128, 6], F32, tag="bias")
            nc.sync.dma_start(bias[:], bias_d.ap())
            ident = pers.tile([128, 128], F32R, tag="ident")
            nc.sync.dma_start(ident[:], ident_d.ap())

            wp = pers.tile([128, 2, NX], BF16, tag="wp")
            nc.sync.dma_start(wp[:], wp_d.ap().rearrange("(c p) n -> p c n", p=128))

            QT = [pers.tile([128, T], BF16, tag=f"qt{i}", name=f"qt{i}")
                  for i in range(2)]
            KTs = [pers.tile([128, T], BF16, tag=f"kt{i}", name=f"kt{i}")
                   for i in range(2)]
            anorm = [pers.tile([128, T], BF16, tag=f"an{i}", name=f"an{i}")
                     for i in range(2)]

            fl_state = {"toggle": 0, "bank": None}

            def fl_bank(shape, dtype=F32):
                fl_state["toggle"] ^= 1
                fl_state["bank"] = bank(6 + fl_state["toggle"], shape, dtype)
                return fl_state["bank"]

            def v_transpose_ops(qq, vstages):
                """Filler ops: PE-transpose V^T chunk -> V natural in vaug."""
                ops = []
                for c2 in range(2):
                    for blk in range(4):
                        def f(c2=c2, blk=blk):
                            vs = vstages[c2]
                            pt_ps = fl_bank([128, 128], F32R)
                            nc.tensor.transpose(
                                pt_ps[:], vs[:, blk * 128:(blk + 1) * 128],
                                ident[:])
                            tt = qq * 4 + blk
                            dst = vaug[:, tt,
                                       c2 * 2 * VW:c2 * 2 * VW + 2 * VW]
                            dst = dst.rearrange("p (h w) -> p h w",
                                                h=2)[:, :, 0:HD]
                            src = pt_ps[:].rearrange("p (h w) -> p h w", h=2)
                            nc.vector.tensor_copy(dst, src)
                        ops.append(f)
                return ops

            def qkv_ops(qq, xt):
                """Filler ops: QKV projection for chunk qq.
                6 groups (q/k/v x c2-half) of 8 accumulating matmuls into a
                toggling filler bank; evac on DVE (with bias); V groups are
                followed by their transpose fillers."""
                cs = slice(qq * QC, (qq + 1) * QC)
                vstages = [None, None]
                ops = []
                for off, kind in ((0, "q"), (CW, "k"), (2 * CW, "v")):
                    for c2 in range(2):
                        for j in range(8):
                            def f(off=off, kind=kind, c2=c2, j=j):
                                if j == 0:
                                    fl_bank([128, QC])
                                g = fl_state["bank"]
                                lhsT = wqkv[:, j,
                                            off + c2 * 128:off + (c2 + 1) * 128]
                                nc.tensor.matmul(g[:], lhsT, xt[:, j, :],
                                                 start=(j == 0), stop=(j == 7))
                                if j == 7:
                                    bcol = {"q": 0, "k": 2, "v": 4}[kind] + c2
                                    bap = bias[:, bcol:bcol + 1]
                                    if kind == "q":
                                        nc.vector.tensor_scalar_add(
                                            QT[c2][:, cs], g[:], bap)
                                    elif kind == "k":
                                        nc.vector.tensor_scalar_add(
                                            KTs[c2][:, cs], g[:], bap)
                                    else:
                                        vs = stg.tile([128, QC], F32R,
                                                      tag="vstage")
                                        nc.vector.tensor_scalar_add(
                                            vs[:], g[:], bap)
                                        vstages[c2] = vs
                            ops.append(f)
                ops += v_transpose_ops(qq, vstages)
                return ops

            def cproj_ops(qq):
                """Filler ops: output projection for t-rows of chunk qq."""
                ops = []
                for i in range(4):
                    tt = qq * 4 + i
                    for nxc in range(2):
                        for c2 in range(2):
                            def f(tt=tt, nxc=nxc, c2=c2, i=i):
                                if c2 == 0:
                                    fl_bank([128, QC])
                                po = fl_state["bank"]
                                nc.tensor.matmul(
                                    po[:],
                                    anorm[c2][:, tt * 128:(tt + 1) * 128],
                                    wp[:, c2, nxc * QC:(nxc + 1) * QC],
                                    start=(c2 == 0), stop=(c2 == 1))
                                if c2 == 1:
                                    ot = op.tile([128, QC], F32, tag="ot")
                                    if (i * 2 + nxc) % 2 == 0:
                                        nc.scalar.copy(ot[:], po[:])
                                    else:
                                        nc.vector.tensor_copy(ot[:], po[:])
                                    nc.sync.dma_start(
                                        out_d.ap()[tt * 128:(tt + 1) * 128,
                                                   nxc * QC:(nxc + 1) * QC],
                                        ot[:])
                            ops.append(f)
                return ops

            def attention_pair(hp, qq, fillers, steps_left):
                """Heads (2hp, 2hp+1) for q-chunk qq, S/PV interleaved with
                filler drain (spread adaptively over remaining steps)."""
                c2 = hp
                nk = 4 * qq + 4
                qs = slice(qq * QC, (qq + 1) * QC)
                pa = [bank(4, [VW, QC]), bank(5, [VW, QC])]
                pts = {}
                LA = 2

                def s_block(kk, hh):
                    ps_s = bank((2 * kk + hh) % 4, [128, QC])
                    rows = slice(64 * hh, 64 * hh + 64)
                    lhsT = KTs[c2][rows, kk * KT:(kk + 1) * KT]
                    rhs = QT[c2][rows, qs]
                    nc.tensor.matmul(ps_s[:], lhsT, rhs, start=True, stop=True)
                    pt = ptp.tile([128, QC], BF16, tag="pt")
                    if kk >= 4 * qq:
                        # band block: columns < 128j are fully masked - skip
                        # their exp; affine_select zero-fills them plus the
                        # above-diagonal triangle of the next 128 columns
                        j = kk - 4 * qq
                        nc.scalar.activation(pt[:, 128 * j:QC],
                                             ps_s[:, 128 * j:QC], Exp,
                                             scale=0.125)
                        if j > 0:
                            nc.gpsimd.memset(pt[:, 0:128 * j], 0.0)
                        nc.gpsimd.affine_select(
                            pt[:, 128 * j:128 * (j + 1)],
                            pt[:, 128 * j:128 * (j + 1)],
                            pattern=[[1, 128]],
                            compare_op=mybir.AluOpType.is_ge, fill=0.0,
                            base=0, channel_multiplier=-1)
                    else:
                        nc.scalar.activation(pt[:], ps_s[:], Exp, scale=0.125)
                    pts[(kk, hh)] = pt

                def pv_block(kk, hh):
                    h = 2 * hp + hh
                    lhsT = vaug[:, kk, h * VW:(h + 1) * VW]
                    nc.tensor.matmul(pa[hh][:], lhsT, pts.pop((kk, hh))[:],
                                     start=(kk == 0), stop=(kk == nk - 1))

                for kk in range(min(LA, nk)):
                    s_block(kk, 0)
                    s_block(kk, 1)
                for kk in range(nk):
                    if kk + LA < nk:
                        s_block(kk + LA, 0)
                        s_block(kk + LA, 1)
                    n = -(-len(fillers) // max(1, steps_left[0]))
                    steps_left[0] -= 1
                    for _ in range(n):
                        if fillers:
                            fillers.pop(0)()
                    pv_block(kk, 0)
                    pv_block(kk, 1)

                for hh in range(2):
                    rows = slice(64 * hh, 64 * hh + 64)
                    dn = rp.tile([1, QC], F32, tag="dn")
                    nc.vector.tensor_copy(dn[:], pa[hh][HD:HD + 1, :])
                    recip = rp.tile([1, QC], F32, tag="recip")
                    nc.vector.reciprocal_approx_fast(recip[:], dn[:])
                    rbc = rp.tile([64, QC], F32, tag="rbc")
                    nc.gpsimd.partition_broadcast(rbc[:], recip[:])
                    nc.vector.tensor_mul(anorm[c2][rows, qs],
                                         pa[hh][0:HD, :], rbc[:])

            # ---- main pipeline over q-chunks ----
            # chunk 0 QKV runs dense up front; each attention stretch then
            # drains the next chunk's QKV + previous chunk's c_proj as
            # fillers between its S and PV matmuls.
            for f in qkv_ops(0, xt0):
                f()
            for qq in range(NQC):
                fillers = []
                if qq + 1 < NQC:
                    fillers += qkv_ops(qq + 1, xts[qq + 1])
                if qq >= 1:
                    fillers += cproj_ops(qq - 1)
                steps_left = [2 * (4 * qq + 4)]
                for hp in range(2):
                    attention_pair(hp, qq, fillers, steps_left)
                while fillers:
                    fillers.pop(0)()
            for f in cproj_ops(NQC - 1):
                f()

    nc.compile()
    return nc


_CACHE = {}


def _get_nc():
    if "nc" not in _CACHE:
        _CACHE["nc"] = _build()
    return _CACHE["nc"]


def kernel(x, w_attn, b_attn, w_proj, b_proj):
    x = np.asarray(x, dtype=np.float32)
    w_attn = np.asarray(w_attn, dtype=np.float32)
    b_attn = np.asarray(b_attn, dtype=np.float32)
    w_proj = np.asarray(w_proj, dtype=np.float32)
    b_proj = np.asarray(b_proj, dtype=np.float32)

    ident = np.eye(128, dtype=np.float32)
    vones = np.ones((128, 64), dtype=NP_BF16)
    in_maps = []
    for core in range(NCORES):
        b, hg = divmod(core, NHG)
        cols = slice(hg * CW, (hg + 1) * CW)
        bias = np.empty((128, 6), dtype=np.float32)
        for qkv_i in range(3):
            bseg = b_attn[qkv_i * NX:][cols]
            bias[:, 2 * qkv_i] = bseg[:128]
            bias[:, 2 * qkv_i + 1] = bseg[128:]
        in_maps.append({
            "xT": np.ascontiguousarray(x[b].T).astype(NP_BF16),
            "wqkv": np.concatenate(
                [w_attn[:, cols], w_attn[:, NX:][:, cols],
                 w_attn[:, 2 * NX:][:, cols]], axis=1).astype(NP_BF16),
            "bias": bias,
            "wp": np.ascontiguousarray(w_proj[cols, :]).astype(NP_BF16),
            "ident": ident,
            "vones": vones,
        })

    nc = _get_nc()
    res = run_bass_kernel_spmd(nc, in_maps, core_ids=list(range(NCORES)))
    _CACHE["last_res"] = res
    out = np.empty((B, T, NX), dtype=np.float32)
    for b in range(B):
        acc = res.results[b * NHG]["out_p"].astype(np.float32)
        for hg in range(1, NHG):
            acc = acc + res.results[b * NHG + hg]["out_p"]
        out[b] = acc + b_proj
    return out


# revision 25
# speedup vs baseline: 1.1358x; 1.0568x over previous
"""Trainium2 Bass kernel for causal multi-head attention (GPT-style block).

Reference computation (per batch b):
    qkv = x @ w_attn + b_attn ; q,k,v = split(qkv)
    per head: S = q k^T / sqrt(64); causal mask; P = softmax(S); a = P v
    out = concat_heads(a) @ w_proj + b_proj

Shapes: x (2, 2048, 1024), 16 heads, head_dim 64.

Sharding: 8 cores = 2 batches x 4 head-groups (4 heads each).  Tensor
parallel over heads: each core computes the QKV projection for its 4 heads
(column slice of w_attn), full causal attention for those heads, and its
partial output projection (row slice of w_proj).  Host sums the 4
head-group partials per batch and adds b_proj.

On-chip layouts (per core, T=2048, CW=256=4*64):
    xT      [1024, T]   x transposed (host-prepped bf16), streamed per chunk
    Q^T,K^T [CW, T]     projections with head-channel on partitions (bf16)
    V_aug   [T, 4*65]   V natural layout + ones column per head (the ones
                        column makes the PV matmul also produce the softmax
                        denominator as output row 64)
    S^T     [k, q]      scores transposed: PSUM [128, 512] per (k-tile,
                        q-chunk); P^T = exp(S^T/8) directly feeds PV as the
                        moving operand - no transposes in the hot loop.

All matmuls are bf16 (1 cycle/row; fp32r measured ~1.85 cyc/row on this
silicon).  Causal masking: off-band blocks are skipped; band blocks are
exp'd only on their live columns and zeroed above the diagonal with
gpsimd.affine_select (exact: exp(s)*0) on the otherwise idle GPSIMD
engine.

The attention inner loop is the only dependency-limited stretch (S ->
exp -> mask -> PV).  To keep the PE dense there (TRN2's HAM re-throttles
the PE clock to 1.2 GHz whenever an activity window contains idle), all
other matmul work - the NEXT chunk's QKV projection groups, V
transposes, and the PREVIOUS chunk's output projection - is queued as
"filler" ops and drained a few per attention step between the S and PV
matmuls.  PSUM banks: 0-3 S-blocks (lookahead 2 x 2 heads), 4/5 PV
accumulators for the head pair, 6/7 toggling filler accumulators.
"""

import sys

sys.path.insert(0, "/opt/trn_rl_repo")

import numpy as np
import ml_dtypes

import concourse.bacc as bacc
import concourse.mybir as mybir
import concourse.tile as tile
from concourse.bass_utils import run_bass_kernel_spmd

F32 = mybir.dt.float32
F32R = mybir.dt.float32r
BF16 = mybir.dt.bfloat16
NP_BF16 = np.dtype(ml_dtypes.bfloat16)

B = 2
T = 2048
NX = 1024
H = 16
HD = 64
NCORES = 8
NHG = 4          # head groups (cores per batch)
NH = 4           # heads per core
CW = NH * HD     # 256 channel width per core
QC = 512         # q-chunk (moving dim)
NQC = T // QC    # 4
KT = 128         # k-tile
VW = HD + 1      # 65: V columns + ones column


def _build():
    nc = bacc.Bacc("TRN2", target_bir_lowering=False, debug=False,
                   num_devices=NCORES)
    xT_d = nc.dram_tensor("xT", [NX, T], BF16, kind="ExternalInput")
    wqkv_d = nc.dram_tensor("wqkv", [NX, 3 * CW], BF16, kind="ExternalInput")
    bias_d = nc.dram_tensor("bias", [128, 6], F32, kind="ExternalInput")
    wp_d = nc.dram_tensor("wp", [CW, NX], BF16, kind="ExternalInput")
    ident_d = nc.dram_tensor("ident", [128, 128], F32R, kind="ExternalInput")
    vones_d = nc.dram_tensor("vones", [128, 16 * NH], BF16, kind="ExternalInput")
    out_d = nc.dram_tensor("out_p", [T, NX], F32, kind="ExternalOutput")

    Exp = mybir.ActivationFunctionType.Exp

    with tile.TileContext(nc) as tc:
        with (
            tc.tile_pool(name="pers", bufs=1) as pers,
            tc.tile_pool(name="xin", bufs=2) as xin,
            tc.tile_pool(name="ps", bufs=1, space="PSUM") as psum,
            tc.tile_pool(name="ptp", bufs=8) as ptp,
            tc.tile_pool(name="stg", bufs=4) as stg,
            tc.tile_pool(name="op", bufs=4) as op,
            tc.tile_pool(name="rp", bufs=4) as rp,
        ):
            def bank(i, shape, dtype=F32):
                return psum.tile(shape, dtype, tag=f"bank{i}", bufs=1,
                                 name=f"bank{i}")

            # ---- persistent tiles; load order: first-needed first ----
            # ones columns of V_aug first: tiny DMA + DVE fill (element-exact;
            # a sub-512B strided DMA would read-modify-write and race the V
            # data copies) so the first PV matmuls never wait on it
            vaug = pers.tile([128, T // KT, NH * VW], BF16, tag="vaug")
            vones_sb = pers.tile([128, 16 * NH], BF16, tag="vones")
            nc.sync.dma_start(vones_sb[:], vones_d.ap())
            nc.vector.tensor_copy(
                vaug[:].rearrange("p t (h w) -> p t h w", h=NH)[:, :, :, HD:HD + 1],
                vones_sb[:].rearrange("p (t h w) -> p t h w", t=16, h=NH),
            )
            bias = pers.tile([128, 6], F32, tag="bias")
            nc.sync.dma_start(bias[:], bias_d.ap())
            ident = pers.tile([128, 128], F32R, tag="ident")
            nc.sync.dma_start(ident[:], ident_d.ap())
            wqkv = pers.tile([128, 8, 3 * CW], BF16, tag="wqkv")
            nc.sync.dma_start(wqkv[:],
                              wqkv_d.ap().rearrange("(j p) c -> p j c", p=128))
            def load_x(qq):
                xt = xin.tile([128, 8, QC], BF16, tag="xt")
                nc.gpsimd.dma_start(
                    xt[:],
                    xT_d.ap().rearrange("(j p) t -> p j t",
                                        p=128)[:, :, qq * QC:(qq + 1) * QC])
                return xt

            xt0 = load_x(0)
            xts = {qq: load_x(qq) for qq in range(1, NQC)}
            bias = pers.tile([128, 6], F32, tag="bias")
            nc.sync.dma_start(bias[:], bias_d.ap())
            ident = pers.tile([128, 128], F32R, tag="ident")
            nc.sync.dma_start(ident[:], ident_d.ap())

            wp = pers.tile([128, 2, NX], BF16, tag="wp")
            nc.sync.dma_start(wp[:], wp_d.ap().rearrange("(c p) n -> p c n", p=128))

            QT = [pers.tile([128, T], BF16, tag=f"qt{i}", name=f"qt{i}")
                  for i in range(2)]
            KTs = [pers.tile([128, T], BF16, tag=f"kt{i}", name=f"kt{i}")
                   for i in range(2)]
            anorm = [pers.tile([128, T], BF16, tag=f"an{i}", name=f"an{i}")
                     for i in range(2)]

            fl_state = {"toggle": 0, "bank": None}

            def fl_bank(shape, dtype=F32):
                fl_state["toggle"] ^= 1
                fl_state["bank"] = bank(6 + fl_state["toggle"], shape, dtype)
                return fl_state["bank"]

            def v_transpose_ops(qq, vstages):
                """Filler ops: PE-transpose V^T chunk -> V natural in vaug."""
                ops = []
                for c2 in range(2):
                    for blk in range(4):
                        def f(c2=c2, blk=blk):
                            vs = vstages[c2]
                            pt_ps = fl_bank([128, 128], F32R)
                            nc.tensor.transpose(
                                pt_ps[:], vs[:, blk * 128:(blk + 1) * 128],
                                ident[:])
                            tt = qq * 4 + blk
                            dst = vaug[:, tt,
                                       c2 * 2 * VW:c2 * 2 * VW + 2 * VW]
                            dst = dst.rearrange("p (h w) -> p h w",
                                                h=2)[:, :, 0:HD]
                            src = pt_ps[:].rearrange("p (h w) -> p h w", h=2)
                            nc.vector.tensor_copy(dst, src)
                        ops.append(f)
                return ops

            def qkv_ops(qq, xt):
                """Filler ops: QKV projection for chunk qq.
                6 groups (q/k/v x c2-half) of 8 accumulating matmuls into a
                toggling filler bank; evac on DVE (with bias); V groups are
                followed by their transpose fillers."""
                cs = slice(qq * QC, (qq + 1) * QC)
                vstages = [None, None]
                ops = []
                for off, kind in ((0, "q"), (CW, "k"), (2 * CW, "v")):
                    for c2 in range(2):
                        for j in range(8):
                            def f(off=off, kind=kind, c2=c2, j=j):
                                if j == 0:
                                    fl_bank([128, QC])
                                g = fl_state["bank"]
                                lhsT = wqkv[:, j,
                                            off + c2 * 128:off + (c2 + 1) * 128]
                                nc.tensor.matmul(g[:], lhsT, xt[:, j, :],
                                                 start=(j == 0), stop=(j == 7))
                                if j == 7:
                                    bcol = {"q": 0, "k": 2, "v": 4}[kind] + c2
                                    bap = bias[:, bcol:bcol + 1]
                                    if kind == "q":
                                        nc.vector.tensor_scalar_add(
                                            QT[c2][:, cs], g[:], bap)
                                    elif kind == "k":
                                        nc.vector.tensor_scalar_add(
                                            KTs[c2][:, cs], g[:], bap)
                                    else:
                                        vs = stg.tile([128, QC], F32R,
                                                      tag="vstage")
                                        nc.vector.tensor_scalar_add(
                                            vs[:], g[:], bap)
                                        vstages[c2] = vs
                            ops.append(f)
                ops += v_transpose_ops(qq, vstages)
                return ops

            def cproj_ops(qq, act_ok=False):
                """Filler ops: output projection for t-rows of chunk qq.
                act_ok: allow ScalarE evacs (final dense chunk only - during
                interleaved chunks the ScalarE paces the attention exps)."""
                ops = []
                for i in range(4):
                    tt = qq * 4 + i
                    for nxc in range(2):
                        for c2 in range(2):
                            def f(tt=tt, nxc=nxc, c2=c2, i=i):
                                if c2 == 0:
                                    fl_bank([128, QC])
                                po = fl_state["bank"]
                                nc.tensor.matmul(
                                    po[:],
                                    anorm[c2][:, tt * 128:(tt + 1) * 128],
                                    wp[:, c2, nxc * QC:(nxc + 1) * QC],
                                    start=(c2 == 0), stop=(c2 == 1))
                                if c2 == 1:
                                    ot = op.tile([128, QC], F32, tag="ot")
                                    if act_ok and (i * 2 + nxc) % 2 == 0:
                                        nc.scalar.copy(ot[:], po[:])
                                    else:
                                        nc.vector.tensor_copy(ot[:], po[:])
                                    nc.sync.dma_start(
                                        out_d.ap()[tt * 128:(tt + 1) * 128,
                                                   nxc * QC:(nxc + 1) * QC],
                                        ot[:])
                            ops.append(f)
                return ops

            def attention_pair(hp, qq, fillers, steps_left):
                """Heads (2hp, 2hp+1) for q-chunk qq, S/PV interleaved with
                filler drain (spread adaptively over remaining steps)."""
                c2 = hp
                nk = 4 * qq + 4
                qs = slice(qq * QC, (qq + 1) * QC)
                pa = [bank(4, [VW, QC]), bank(5, [VW, QC])]
                pts = {}
                LA = 2

                def s_block(kk, hh):
                    ps_s = bank((2 * kk + hh) % 4, [128, QC])
                    rows = slice(64 * hh, 64 * hh + 64)
                    lhsT = KTs[c2][rows, kk * KT:(kk + 1) * KT]
                    rhs = QT[c2][rows, qs]
                    nc.tensor.matmul(ps_s[:], lhsT, rhs, start=True, stop=True)
                    pt = ptp.tile([128, QC], BF16, tag="pt")
                    if kk >= 4 * qq:
                        # band block: columns < 128j are fully masked - skip
                        # their exp; affine_select zero-fills them plus the
                        # above-diagonal triangle of the next 128 columns
                        j = kk - 4 * qq
                        nc.scalar.activation(pt[:, 128 * j:QC],
                                             ps_s[:, 128 * j:QC], Exp,
                                             scale=0.125)
                        if j > 0:
                            nc.gpsimd.memset(pt[:, 0:128 * j], 0.0)
                        nc.gpsimd.affine_select(
                            pt[:, 128 * j:128 * (j + 1)],
                            pt[:, 128 * j:128 * (j + 1)],
                            pattern=[[1, 128]],
                            compare_op=mybir.AluOpType.is_ge, fill=0.0,
                            base=0, channel_multiplier=-1)
                    else:
                        nc.scalar.activation(pt[:], ps_s[:], Exp, scale=0.125)
                    pts[(kk, hh)] = pt

                def pv_block(kk, hh):
                    h = 2 * hp + hh
                    lhsT = vaug[:, kk, h * VW:(h + 1) * VW]
                    nc.tensor.matmul(pa[hh][:], lhsT, pts.pop((kk, hh))[:],
                                     start=(kk == 0), stop=(kk == nk - 1))

                for kk in range(min(LA, nk)):
                    s_block(kk, 0)
                    s_block(kk, 1)
                for kk in range(nk):
                    if kk + LA < nk:
                        s_block(kk + LA, 0)
                        s_block(kk + LA, 1)
                    n = -(-len(fillers) // max(1, steps_left[0]))
                    steps_left[0] -= 1
                    for _ in range(n):
                        if fillers:
                            fillers.pop(0)()
                    pv_block(kk, 0)
                    pv_block(kk, 1)

                for hh in range(2):
                    rows = slice(64 * hh, 64 * hh + 64)
                    dn = rp.tile([1, QC], F32, tag="dn")
                    nc.vector.tensor_copy(dn[:], pa[hh][HD:HD + 1, :])
                    recip = rp.tile([1, QC], F32, tag="recip")
                    nc.vector.reciprocal_approx_fast(recip[:], dn[:])
                    rbc = rp.tile([64, QC], F32, tag="rbc")
                    nc.gpsimd.partition_broadcast(rbc[:], recip[:])
                    nc.vector.tensor_mul(anorm[c2][rows, qs],
                                         pa[hh][0:HD, :], rbc[:])

            # ---- main pipeline over q-chunks ----
            # chunk 0 QKV runs dense up front; each attention stretch then
            # drains the next chunk's QKV + previous chunk's c_proj as
            # fillers between its S and PV matmuls.
            for f in qkv_ops(0, xt0):
                f()
            for qq in range(NQC):
                fillers = []
                if qq + 1 < NQC:
                    fillers += qkv_ops(qq + 1, load_x(qq + 1))
                if qq >= 1:
                    fillers += cproj_ops(qq - 1)
                steps_left = [2 * (4 * qq + 4)]
                for hp in range(2):
                    attention_pair(hp, qq, fillers, steps_left)
                while fillers:
                    fillers.pop(0)()
            for f in cproj_ops(NQC - 1, act_ok=True):
                f()

    nc.compile()
    return nc


_CACHE = {}


def _get_nc():
    if "nc" not in _CACHE:
        _CACHE["nc"] = _build()
    return _CACHE["nc"]


def kernel(x, w_attn, b_attn, w_proj, b_proj):
    x = np.asarray(x, dtype=np.float32)
    w_attn = np.asarray(w_attn, dtype=np.float32)
    b_attn = np.asarray(b_attn, dtype=np.float32)
    w_proj = np.asarray(w_proj, dtype=np.float32)
    b_proj = np.asarray(b_proj, dtype=np.float32)

    ident = np.eye(128, dtype=np.float32)
    vones = np.ones((128, 64), dtype=NP_BF16)
    in_maps = []
    for core in range(NCORES):
        b, hg = divmod(core, NHG)
        cols = slice(hg * CW, (hg + 1) * CW)
        bias = np.empty((128, 6), dtype=np.float32)
        for qkv_i in range(3):
            bseg = b_attn[qkv_i * NX:][cols]
            bias[:, 2 * qkv_i] = bseg[:128]
            bias[:, 2 * qkv_i + 1] = bseg[128:]
        in_maps.append({
            "xT": np.ascontiguousarray(x[b].T).astype(NP_BF16),
            "wqkv": np.concatenate(
                [w_attn[:, cols], w_attn[:, NX:][:, cols],
                 w_attn[:, 2 * NX:][:, cols]], axis=1).astype(NP_BF16),
            "bias": bias,
            "wp": np.ascontiguousarray(w_proj[cols, :]).astype(NP_BF16),
            "ident": ident,
            "vones": vones,
        })

    nc = _get_nc()
    res = run_bass_kernel_spmd(nc, in_maps, core_ids=list(range(NCORES)))
    _CACHE["last_res"] = res
    out = np.empty((B, T, NX), dtype=np.float32)
    for b in range(B):
        acc = res.results[b * NHG]["out_p"].astype(np.float32)
        for hg in range(1, NHG):
            acc = acc + res.results[b * NHG + hg]["out_p"]
        out[b] = acc + b_proj
    return out
